# revision 1
# baseline (speedup 1.0000x reference)
"""Trainium2 Bass kernel for nn_CirModel_52956946760343 (14-qubit circuit sim).

Formulation: per-sample state = 128x128 complex matrix (rows = qubits 0-6,
cols = qubits 7-13). Each rotation layer = A_l S B_l^T with A_l, B_l 128x128
Kronecker products of per-qubit 2x2 unitaries (host-computed from `weight`).
CNOT rings fold into: host-side permutations of those matrices, dual-class
stationary variants (crossing CNOT conditioned on the free axis), and
PSUM-evict access patterns (crossing CNOT conditioned on partitions).
The final layer's column gate drops (Z_0 observable is invariant), and the
measurement is a sign-weighted sum of squares via ScalarE accum_out + one
reduce matmul. 9 matmul passes over the state total.

Sharding: pure data parallel, 64 samples per core on 8 cores.
"""

import os

import numpy as np

N_CORES = 8
B_FULL = 512
S = B_FULL // N_CORES  # samples per core
GROUPS = S // 4
N_QUBITS = 14
W_MUL = (2.0**0.5) * (5.0**-0.5)
I128 = np.arange(128)

# matmul operand dtype: "float16" (2-byte state, int32-packed gpsimd
# swizzles), "float32r" (full-rate fp32 variant) or "float32"
MM_DTYPE = os.environ.get("CIR_MM_DTYPE", "float16")
# debug/attribution knobs (TimelineSim experiments only)
SKIP_COPIES = False
SKIP_MMS = False
INTERLEAVE = os.environ.get("CIR_INTERLEAVE", "0") == "1"
# P2 evict: contiguous psum->scratch on DVE/Act + gpsimd swizzle placement
GPS = os.environ.get("CIR_GPS", "1") == "1"

# ----------------------------------------------------------------- host plan


def _rx(t):
    c, s = np.cos(t / 2), np.sin(t / 2)
    return np.array([[c, -1j * s], [-1j * s, c]], dtype=np.complex128)


def _ry(t):
    c, s = np.cos(t / 2), np.sin(t / 2)
    return np.array([[c, -s], [s, c]], dtype=np.complex128)


def _rz(t):
    e = np.exp(-0.5j * t)
    return np.array([[e, 0], [0, np.conj(e)]], dtype=np.complex128)


_LAYER_GATES = [
    (_rx, _ry, _rz),
    (_ry, _rz, _ry),
    (_rz, _ry, _rx),
    (_rx, _rz, _rx),
    (_ry, _rz, _ry),
]


def _layer_group_mats(weight):
    w = np.asarray(weight, dtype=np.float64) * W_MUL
    A, Bm = [], []
    for li, gates in enumerate(_LAYER_GATES):
        us = []
        for i in range(N_QUBITS):
            u = np.eye(2, dtype=np.complex128)
            for j, g in enumerate(gates):
                u = g(w[42 * li + 3 * i + j]) @ u
            us.append(u)
        m = us[0]
        for u in us[1:7]:
            m = np.kron(m, u)
        A.append(m)
        m = us[7]
        for u in us[8:]:
            m = np.kron(m, u)
        Bm.append(m)
    return A, Bm


def _cnot_chain_perm():
    def fn(x):
        bits = [(x >> (6 - i)) & 1 for i in range(7)]
        for c, t in [(i, i + 1) for i in range(6)]:
            bits[t] ^= bits[c]
        y = 0
        for i in range(7):
            y = (y << 1) | bits[i]
        return y

    return np.array([fn(x) for x in range(128)], dtype=np.int64)


_P_R = _cnot_chain_perm()
_P_C = _P_R.copy()
_INV_PR = np.argsort(_P_R)
_INV_PC = np.argsort(_P_C)
_FLIP_MSB = I128 ^ 64
_BIT_LSB = I128 & 1
_MU = _P_C & 1  # parity of all 7 bits


def _order_rows_ring():
    return (I128 & 1) * 64 + (I128 >> 1)


def _order_cols_even_ring():
    return (I128 & 1) * 64 + (I128 >> 1)


def _order_cols_odd_ring():
    b7 = (I128 >> 6) & 1
    rest = I128 & 63
    mutilde = np.array([bin(int(r)).count("1") & 1 for r in rest])
    return b7 * 64 + mutilde * 32 + (rest >> 1)


_ORDER_V = ((I128 >> 6) & 1) * 64 + (I128 & 63)

# const-tile column offsets
_OFF_P1 = [i * 512 for i in range(4)]  # [A(256) | B(256)] per P1 pass
_OFF_P2 = [[2048 + (i * 2 + k) * 384 for k in (0, 1)] for i in range(4)]
_OFF_V = [5120 + k * 384 for k in (0, 1)]
_OFF_SIGN = 5888
_NCONST = 5890


def _pack_lhst(dst, off, L):
    """Pack [LiT | LrT | nLiT] (each 128 cols) for complex matmul L @ X."""
    dst[:, off : off + 128] = L.imag.T
    dst[:, off + 128 : off + 256] = L.real.T
    dst[:, off + 256 : off + 384] = -L.imag.T


def build_plan_consts(weight):
    """Returns the [128, _NCONST] float32 const tensor."""
    A, Bm = _layer_group_mats(weight)
    C = np.zeros((128, _NCONST), dtype=np.float64)

    cur_row_pos = I128.copy()
    cur_col_pos = I128.copy()
    pending_col = I128.copy()

    for l in range(4):
        if l % 2 == 0:
            new_row_pos = _order_rows_ring()
            lab_p = np.argsort(cur_row_pos)
            lab_j = np.argsort(new_row_pos)
            R = A[l][_INV_PR[lab_j][:, None], lab_p[None, :]].T
            C[:, _OFF_P1[l] : _OFF_P1[l] + 128] = R.real
            C[:, _OFF_P1[l] + 128 : _OFF_P1[l] + 256] = R.imag
            C[:, _OFF_P1[l] + 256 : _OFF_P1[l] + 384] = -R.imag
            C[:, _OFF_P1[l] + 384 : _OFF_P1[l] + 512] = R.real
            cur_row_pos = new_row_pos

            new_col_pos = _order_cols_even_ring()
            lab_p = np.argsort(cur_col_pos)
            lab_q = np.argsort(new_col_pos)
            in_idx = pending_col[lab_p]
            L0 = Bm[l][_INV_PC[lab_q][:, None], in_idx[None, :]]
            L1 = Bm[l][_FLIP_MSB[_INV_PC[lab_q]][:, None], in_idx[None, :]]
            _pack_lhst(C, _OFF_P2[l][0], L0)
            _pack_lhst(C, _OFF_P2[l][1], L1)
            cur_col_pos = new_col_pos
            pending_col = I128.copy()
        else:
            new_col_pos = _order_cols_odd_ring()
            lab_p = np.argsort(cur_col_pos)
            lab_j = np.argsort(new_col_pos)
            R = Bm[l][lab_j[:, None], lab_p[None, :]].T
            C[:, _OFF_P1[l] : _OFF_P1[l] + 128] = R.real
            C[:, _OFF_P1[l] + 128 : _OFF_P1[l] + 256] = R.imag
            C[:, _OFF_P1[l] + 256 : _OFF_P1[l] + 384] = -R.imag
            C[:, _OFF_P1[l] + 384 : _OFF_P1[l] + 512] = R.real
            cur_col_pos = new_col_pos

            new_row_pos = _order_rows_ring()
            lab_p = np.argsort(cur_row_pos)
            lab_q = np.argsort(new_row_pos)
            if l == 3:
                L0 = A[l][_INV_PR[lab_q][:, None], lab_p[None, :]]
                L1 = L0
            else:
                flip_mu0 = np.where(_BIT_LSB == 1, _FLIP_MSB, I128)
                flip_mu1 = np.where(_BIT_LSB == 0, _FLIP_MSB, I128)
                L0 = A[l][_INV_PR[flip_mu0[lab_q]][:, None], lab_p[None, :]]
                L1 = A[l][_INV_PR[flip_mu1[lab_q]][:, None], lab_p[None, :]]
            _pack_lhst(C, _OFF_P2[l][0], L0)
            _pack_lhst(C, _OFF_P2[l][1], L1)
            cur_row_pos = new_row_pos
            pending_col = _P_C.copy()

    lab_p = np.argsort(cur_row_pos)
    lab_q = np.argsort(_ORDER_V)
    V0 = A[4][lab_q[:, None], lab_p[None, :]]
    V1 = A[4][lab_q[:, None], _FLIP_MSB[lab_p][None, :]]
    _pack_lhst(C, _OFF_V[0], V0)
    _pack_lhst(C, _OFF_V[1], V1)

    C[:64, _OFF_SIGN] = 1.0
    C[64:, _OFF_SIGN] = -1.0
    return np.ascontiguousarray(C, dtype=np.float32)


# ------------------------------------------------------------- bass program

_PROGRAM_CACHE = {}


def build_program(
    n_samples=S,
    mm_dtype_name=MM_DTYPE,
    n_cores=N_CORES,
    debug_after=None,
    repeat=1,
):
    import concourse.bass as bass
    import concourse.mybir as mybir
    import concourse.tile as tile
    from concourse import bacc

    DT = getattr(mybir.dt, mm_dtype_name)
    F32 = mybir.dt.float32
    I32 = mybir.dt.int32
    pack2 = mm_dtype_name == "float16"  # 2-byte state: int32-pack swizzles
    ng = n_samples // 8  # 8-sample groups

    nc = bacc.Bacc(
        "TRN2",
        target_bir_lowering=False,
        debug=False,
        enable_asserts=False,
        num_devices=n_cores,
    )

    xr_d = nc.dram_tensor("xr", [n_samples, 16384], DT, kind="ExternalInput").ap()
    xi_d = nc.dram_tensor("xi", [n_samples, 16384], DT, kind="ExternalInput").ap()
    c_d = nc.dram_tensor("consts", [128, _NCONST], DT, kind="ExternalInput").ap()
    out_d = nc.dram_tensor("out", [n_samples, 1], F32, kind="ExternalOutput").ap()

    FREE = n_samples * 128
    if debug_after is not None:
        dbg_r = nc.dram_tensor(
            "dbg_r", [128, FREE], DT, kind="ExternalOutput"
        ).ap()
        dbg_i = nc.dram_tensor(
            "dbg_i", [128, FREE], DT, kind="ExternalOutput"
        ).ap()

    with tile.TileContext(nc) as tc:
        import contextlib

        ctx = contextlib.ExitStack()
        with ctx:
            singles = ctx.enter_context(tc.tile_pool(name="singles", bufs=1))
            psp = ctx.enter_context(tc.tile_pool(name="ps", bufs=4, space="PSUM"))
            scrp = ctx.enter_context(tc.tile_pool(name="scr", bufs=3))

            ctile = singles.tile([128, _NCONST], DT, tag="consts", name="ctile")
            # combined state: [Re | Im] per buffer
            X = [
                singles.tile([128, 2 * FREE], DT, tag="x0", name="x0"),
                singles.tile([128, 2 * FREE], DT, tag="x1", name="x1"),
            ]
            asum = singles.tile([128, n_samples], F32, tag="asum", name="asum")
            obuf = singles.tile([64, 1], F32, tag="obuf", name="obuf")
            sgn = singles.tile([128, 1], F32, tag="sgn", name="sgn")
            nc.gpsimd.memset(sgn[0:64], 1.0)
            nc.gpsimd.memset(sgn[64:128], -1.0)

            def planes(k):
                return X[k][:, :FREE], X[k][:, FREE:]

            # ---- load constants + input state
            nc.sync.dma_start(ctile[:], c_d)
            chunk = 8 if n_samples % 8 == 0 else 4
            X0r, X0i = planes(0)
            for c0 in range(0, n_samples, chunk):
                for src_d, dstp in ((xr_d, X0r), (xi_d, X0i)):
                    src = src_d[c0 : c0 + chunk, :].rearrange(
                        "s (r f) -> r s f", f=128
                    )
                    dst = dstp[:, c0 * 128 : (c0 + chunk) * 128].rearrange(
                        "p (s f) -> p s f", f=128
                    )
                    nc.sync.dma_start(dst, src)

            def mm(out_ap, lhsT, rhs, start, stop):
                if SKIP_MMS:
                    return
                nc.tensor.matmul(
                    out_ap,
                    lhsT.bitcast(DT),
                    rhs.bitcast(DT),
                    start=start,
                    stop=stop,
                )

            cp_engines = [
                lambda o, i: nc.vector.tensor_copy(o, i),
                lambda o, i: nc.scalar.copy(o, i),
            ]
            cp_idx = [0]

            def cp(o, i):
                if SKIP_COPIES:
                    return
                cp_engines[cp_idx[0] % 2](o, i)
                cp_idx[0] += 1

            def emit_p1_group(li, ksrc, kdst, g):
                sR, sI = planes(ksrc)
                base = _OFF_P1[li]
                cA = ctile[:, base : base + 256]
                cB = ctile[:, base + 256 : base + 512]
                if True:
                    ps = psp.tile([128, 1024], F32, tag="grp", name="psgrp")
                    for j in range(4):
                        s = 4 * g + j
                        sl = ps[:, 256 * j : 256 * j + 256]
                        mm(sl, sR[:, s * 128 : (s + 1) * 128], cA, True, False)
                        mm(sl, sI[:, s * 128 : (s + 1) * 128], cB, False, True)
                    # one merged-plane evict: psum (s, pl, 128) -> dst (pl, s, 128)
                    pv = ps[:].rearrange("p (s pl f) -> p pl s f", pl=2, f=128)
                    dv = X[kdst][:].rearrange(
                        "p (pl s f) -> p pl s f", pl=2, f=128
                    )[:, :, 4 * g : 4 * g + 4, :]
                    cp(dv, pv)

            def class_rhs(view_f128, view_x4, g, parity, k):
                if parity == 0:
                    return view_f128[:, 8 * g : 8 * g + 8, 64 * k : 64 * k + 64]
                if k == 0:
                    return view_x4[:, 8 * g : 8 * g + 8, ::3, :]
                return view_f128[:, 8 * g : 8 * g + 8, 32:96]

            def emit_p2_mms_class(ps, offs2, ksrc, g, parity, k):
                """One class-tile: ps = [ReC(512) | ImC(512)]. For the
                single-class case (parity-1 l=3), k indexes group halves."""
                sR, sI = planes(ksrc)
                if parity == 1 and offs2[0] == offs2[1]:
                    base = offs2[0]
                    sl = slice(1024 * g + 512 * k, 1024 * g + 512 * k + 512)
                    rr = sR[:, sl]
                    ri = sI[:, sl]
                else:
                    rf = [
                        v.rearrange("p (s f) -> p s f", f=128) for v in (sR, sI)
                    ]
                    r4 = [
                        v.rearrange("p (s x y) -> p s x y", x=4, y=32)
                        for v in (sR, sI)
                    ]
                    base = offs2[k]
                    rr = class_rhs(rf[0], r4[0], g, parity, k)
                    ri = class_rhs(rf[1], r4[1], g, parity, k)
                LiT = ctile[:, base : base + 128]
                LrT = ctile[:, base + 128 : base + 256]
                nLiT = ctile[:, base + 256 : base + 384]
                re_sl = ps[:, 0:512]
                im_sl = ps[:, 512:1024]
                mm(im_sl, LiT, rr, True, False)
                mm(im_sl, LrT, ri, False, True)
                mm(re_sl, LrT, rr, True, False)
                mm(re_sl, nLiT, ri, False, True)

            def evict_p2_class(ps, g, parity, nclass, kdst, k):
                # ps: [ReC(8s x 64) | ImC(8s x 64)] for class k (or half k
                # of the single-class l=3 case).
                gs = slice(8 * g, 8 * g + 8)
                D = X[kdst][:].rearrange("p (pl s f) -> p pl s f", pl=2, f=128)[
                    :, :, gs, :
                ]
                D4 = X[kdst][:].rearrange(
                    "p (pl s x y) -> p pl s x y", pl=2, x=4, y=32
                )[:, :, gs, :, :]
                pv = ps[:].rearrange("p (pl s f) -> p pl s f", pl=2, f=64)
                ph = ps[:].rearrange(
                    "p (pl s x y) -> p pl s x y", pl=2, x=2, y=32
                )
                if nclass == 1:
                    # half k covers samples 4k..4k+3 of the group, full 128 f
                    hs = slice(8 * g + 4 * k, 8 * g + 4 * k + 4)
                    Dh = X[kdst][:].rearrange(
                        "p (pl s f) -> p pl s f", pl=2, f=128
                    )[:, :, hs, :]
                    d2 = X[kdst][:].rearrange(
                        "p (pl s x y) -> p pl s x y", pl=2, x=2, y=64
                    )[:, :, hs, :, :]
                    pvh = ps[:].rearrange("p (pl s f) -> p pl s f", pl=2, f=128)
                    p2 = ps[:].rearrange(
                        "p (pl s x y) -> p pl s x y", pl=2, x=2, y=64
                    )
                    cp(Dh[0:64], pvh[0:64])
                    cp(d2[64:128, :, :, 1, :], p2[64:128, :, :, 0, :])
                    cp(d2[64:128, :, :, 0, :], p2[64:128, :, :, 1, :])
                    return
                if parity == 0:
                    cp(D[0:64, :, :, 64 * k : 64 * k + 64], pv[0:64])
                    cp(D4[64:128, :, :, 2 * k + 1, :], ph[64:128, :, :, 0, :])
                    cp(D4[64:128, :, :, 2 * k, :], ph[64:128, :, :, 1, :])
                else:
                    if k == 0:
                        cp(D4[0:64, :, :, ::3, :], ph[0:64])
                        cp(D4[64:128, :, :, 2, :], ph[64:128, :, :, 0, :])
                        cp(D4[64:128, :, :, 1, :], ph[64:128, :, :, 1, :])
                    else:
                        cp(D[0:64, :, :, 32:96], pv[0:64])
                        cp(D4[64:128, :, :, 3, :], ph[64:128, :, :, 0, :])
                        cp(D4[64:128, :, :, 0, :], ph[64:128, :, :, 1, :])

            def gcp(o, i):
                # gpsimd swizzle; int32-pack halves the element count for
                # 2-byte state dtypes (innermost runs are 32+ elems, 4B-aligned)
                if pack2:
                    o, i = o.bitcast(I32), i.bitcast(I32)
                nc.gpsimd.tensor_copy(o, i)

            def gps_place_class(scr, g, parity, nclass, kdst, k):
                # same placement patterns as evict_p2_class, but reading the
                # scratch copy and with the upper-half block swaps folded into
                # single negative-step views.
                gs = slice(8 * g, 8 * g + 8)
                D = X[kdst][:].rearrange("p (pl s f) -> p pl s f", pl=2, f=128)[
                    :, :, gs, :
                ]
                D4 = X[kdst][:].rearrange(
                    "p (pl s x y) -> p pl s x y", pl=2, x=4, y=32
                )[:, :, gs, :, :]
                sc = scr[:, 1024 * k : 1024 * k + 1024]
                sv = sc.rearrange("p (pl s f) -> p pl s f", pl=2, f=64)
                sh = sc.rearrange("p (pl s x y) -> p pl s x y", pl=2, x=2, y=32)
                if nclass == 1:
                    hs = slice(8 * g + 4 * k, 8 * g + 4 * k + 4)
                    Dh = X[kdst][:].rearrange(
                        "p (pl s f) -> p pl s f", pl=2, f=128
                    )[:, :, hs, :]
                    d2 = X[kdst][:].rearrange(
                        "p (pl s x y) -> p pl s x y", pl=2, x=2, y=64
                    )[:, :, hs, :, :]
                    svh = sc.rearrange("p (pl s f) -> p pl s f", pl=2, f=128)
                    s2 = sc.rearrange("p (pl s x y) -> p pl s x y", pl=2, x=2, y=64)
                    gcp(Dh[0:64], svh[0:64])
                    # dst x (1,0) <- src x (0,1)
                    gcp(d2[64:128, :, :, 1::-1, :], s2[64:128])
                    return
                if parity == 0:
                    gcp(D[0:64, :, :, 64 * k : 64 * k + 64], sv[0:64])
                    # dst x (2k+1, 2k) <- src x (0, 1)
                    xs = slice(2 * k + 1, (2 * k - 1) if k > 0 else None, -1)
                    gcp(D4[64:128, :, :, xs, :], sh[64:128])
                else:
                    if k == 0:
                        gcp(D4[0:64, :, :, ::3, :], sh[0:64])
                        # dst x (2,1) <- src x (0,1)
                        gcp(D4[64:128, :, :, 2:0:-1, :], sh[64:128])
                    else:
                        gcp(D[0:64, :, :, 32:96], sv[0:64])
                        # dst x (3,0) <- src x (0,1)
                        gcp(D4[64:128, :, :, 3::-3, :], sh[64:128])

            def emit_p2_group(li, ksrc, kdst, g):
                parity = li % 2
                nclass = 1 if li == 3 else 2
                offs2 = _OFF_P2[li]
                if nclass == 1:
                    offs2 = [offs2[0], offs2[0]]
                if GPS:
                    scr = scrp.tile([128, 2048], DT, tag="scr", name="scr")
                    for k in range(2):
                        ps = psp.tile([128, 1024], F32, tag="grp", name="psgrp")
                        emit_p2_mms_class(ps, offs2, ksrc, g, parity, k)
                        cp(scr[:, 1024 * k : 1024 * k + 1024], ps[:])
                    for k in range(2):
                        gps_place_class(scr, g, parity, nclass, kdst, k)
                    return
                for k in range(2):
                    ps = psp.tile([128, 1024], F32, tag="grp", name="psgrp")
                    emit_p2_mms_class(ps, offs2, ksrc, g, parity, k)
                    evict_p2_class(ps, g, parity, nclass, kdst, k)

            def emit_p1(li, ksrc, kdst):
                for g in range(2 * ng):
                    emit_p1_group(li, ksrc, kdst, g)

            def emit_p2(li, ksrc, kdst):
                for g in range(ng):
                    emit_p2_group(li, ksrc, kdst, g)

            def emit_layer_interleaved(li, a, b):
                for g in range(ng):
                    emit_p1_group(li, a, b, 2 * g)
                    emit_p1_group(li, a, b, 2 * g + 1)
                    emit_p2_group(li, b, a, g)

            # ---- the 8 layer passes (+ V), optionally in a HW loop
            state = {"done": 0, "tail": True}
            stop = 9 if debug_after is None else debug_after

            def emit_main():
                done = state["done"]
                for li in range(4):
                    if INTERLEAVE and done + 2 <= stop:
                        emit_layer_interleaved(li, 0, 1)
                        done += 2
                        continue
                    if done < stop:
                        emit_p1(li, 0, 1)
                        done += 1
                        if done == stop:
                            nc.sync.dma_start(dbg_r, X[1][:, :FREE])
                            nc.sync.dma_start(dbg_i, X[1][:, FREE:])
                    if done < stop:
                        emit_p2(li, 1, 0)
                        done += 1
                        if done == stop:
                            nc.sync.dma_start(dbg_r, X[0][:, :FREE])
                            nc.sync.dma_start(dbg_i, X[0][:, FREE:])
                tail = done < stop
                # ---- V pass (reads buf 0, squares into buf 1 scratch)
                sv2 = X[1][:].rearrange(
                    "p (s k2 f) -> p k2 s f", k2=4, f=64
                )
                for g in range(ng if tail else 0):
                    for k in range(2):
                        ps = psp.tile([128, 1024], F32, tag="grp", name="psgrp")
                        emit_p2_mms_class(ps, _OFF_V, 0, g, parity=1, k=k)
                        # evict = square: ps (pl, s, 64) -> scratch k2-slots
                        pv = ps[:].rearrange("p (pl s f) -> p pl s f", pl=2, f=64)
                        nc.scalar.activation(
                            sv2[:, k::2, 8 * g : 8 * g + 8, :],
                            pv,
                            bass.mybir.ActivationFunctionType.Square,
                        )
                if tail:
                    xq = X[1][:].rearrange("p (s q) -> p s q", q=256)
                    for g in range(ng):
                        gs = slice(8 * g, 8 * g + 8)
                        nc.vector.tensor_reduce(
                            asum[:, gs].unsqueeze(-1),
                            xq[:, gs, :],
                            axis=bass.mybir.AxisListType.X,
                            op=bass.mybir.AluOpType.add,
                        )
                state["done"] = done
                state["tail"] = tail

            if repeat > 1:
                with tc.For_i(0, repeat, 1):
                    emit_main()
            else:
                emit_main()
            tail = state["tail"]

            # ---- final signed reduce: out[s] = sum_p sign[p] * asum[p, s]
            if tail:
                psfin = psp.tile([n_samples, 1], F32, tag="grp", name="psfin")
                nc.tensor.matmul(
                    psfin[:],
                    asum[:].bitcast(F32),
                    sgn[:].bitcast(F32),
                    start=True,
                    stop=True,
                )
                nc.vector.tensor_copy(obuf[0:n_samples], psfin[:])
                nc.sync.dma_start(out_d, obuf[0:n_samples])
            else:
                nc.vector.memset(obuf[:], 0.0)
                nc.sync.dma_start(out_d, obuf[0:n_samples])

    nc.compile()
    return nc


# ----------------------------------------------------------------- entry


def round_fp32r(a):
    """Round-to-nearest-even to 11 mantissa bits (FP32R)."""
    u = np.ascontiguousarray(a, dtype=np.float32).view(np.uint32)
    r = (u + np.uint32(0x7FF) + ((u >> np.uint32(12)) & np.uint32(1))) & np.uint32(
        0xFFFFF000
    )
    return r.view(np.float32)


def kernel(x_real, x_imag, weight):
    from concourse.bass_utils import run_bass_kernel_spmd

    key = (S, MM_DTYPE, N_CORES)
    if key not in _PROGRAM_CACHE:
        _PROGRAM_CACHE[key] = build_program(S, MM_DTYPE, N_CORES)
    nc = _PROGRAM_CACHE[key]

    consts = build_plan_consts(np.asarray(weight, dtype=np.float64))
    xr = np.ascontiguousarray(
        np.asarray(x_real, dtype=np.float32).reshape(B_FULL, 16384)
    )
    xi = np.ascontiguousarray(
        np.asarray(x_imag, dtype=np.float32).reshape(B_FULL, 16384)
    )
    if MM_DTYPE == "float16":
        consts = consts.astype(np.float16)
        xr = xr.astype(np.float16)
        xi = xi.astype(np.float16)
    if MM_DTYPE == "float32r":
        consts = round_fp32r(consts)
        xr = round_fp32r(xr)
        xi = round_fp32r(xi)

    in_maps = []
    for c in range(N_CORES):
        in_maps.append(
            {
                "xr": xr[c * S : (c + 1) * S],
                "xi": xi[c * S : (c + 1) * S],
                "consts": consts,
            }
        )

    trace = os.environ.get("CIR_TRACE", "") == "1"
    res = run_bass_kernel_spmd(
        nc, in_maps, core_ids=list(range(N_CORES)), trace=trace
    )
    LAST_RESULTS[0] = res
    out = np.concatenate([r["out"] for r in res.results], axis=0)
    return out.astype(np.float32)


LAST_RESULTS = [None]



# revision 2
# speedup vs baseline: 3.0418x; 3.0418x over previous
"""Trainium2 Bass kernel for nn_CirModel_52956946760343 (14-qubit circuit sim).

Formulation: per-sample state = 128x128 complex matrix (rows = qubits 0-6,
cols = qubits 7-13). Each rotation layer = A_l S B_l^T with A_l, B_l 128x128
Kronecker products of per-qubit 2x2 unitaries (host-computed from `weight`).
CNOT rings fold into: host-side permutations of those matrices, dual-class
stationary variants (crossing CNOT conditioned on the free axis), and
PSUM-evict access patterns (crossing CNOT conditioned on partitions).
The final layer's column gate drops (Z_0 observable is invariant), and the
measurement is a sign-weighted sum of squares via ScalarE accum_out + one
reduce matmul. 9 matmul passes over the state total.

Sharding: pure data parallel, 64 samples per core on 8 cores.
"""

import os

import numpy as np

N_CORES = 8
B_FULL = 512
S = B_FULL // N_CORES  # samples per core
GROUPS = S // 4
N_QUBITS = 14
W_MUL = (2.0**0.5) * (5.0**-0.5)
I128 = np.arange(128)

# matmul operand dtype: "float16" (2-byte state, int32-packed gpsimd
# swizzles), "float32r" (full-rate fp32 variant) or "float32"
MM_DTYPE = os.environ.get("CIR_MM_DTYPE", "float16")
# debug/attribution knobs (TimelineSim experiments only)
SKIP_COPIES = False
SKIP_MMS = False
INTERLEAVE = os.environ.get("CIR_INTERLEAVE", "0") == "1"
# P2 evict: contiguous psum->scratch on DVE/Act + gpsimd swizzle placement
GPS = os.environ.get("CIR_GPS", "0") == "1"

# ----------------------------------------------------------------- host plan


def _rx(t):
    c, s = np.cos(t / 2), np.sin(t / 2)
    return np.array([[c, -1j * s], [-1j * s, c]], dtype=np.complex128)


def _ry(t):
    c, s = np.cos(t / 2), np.sin(t / 2)
    return np.array([[c, -s], [s, c]], dtype=np.complex128)


def _rz(t):
    e = np.exp(-0.5j * t)
    return np.array([[e, 0], [0, np.conj(e)]], dtype=np.complex128)


_LAYER_GATES = [
    (_rx, _ry, _rz),
    (_ry, _rz, _ry),
    (_rz, _ry, _rx),
    (_rx, _rz, _rx),
    (_ry, _rz, _ry),
]


def _layer_group_mats(weight):
    w = np.asarray(weight, dtype=np.float64) * W_MUL
    A, Bm = [], []
    for li, gates in enumerate(_LAYER_GATES):
        us = []
        for i in range(N_QUBITS):
            u = np.eye(2, dtype=np.complex128)
            for j, g in enumerate(gates):
                u = g(w[42 * li + 3 * i + j]) @ u
            us.append(u)
        m = us[0]
        for u in us[1:7]:
            m = np.kron(m, u)
        A.append(m)
        m = us[7]
        for u in us[8:]:
            m = np.kron(m, u)
        Bm.append(m)
    return A, Bm


def _cnot_chain_perm():
    def fn(x):
        bits = [(x >> (6 - i)) & 1 for i in range(7)]
        for c, t in [(i, i + 1) for i in range(6)]:
            bits[t] ^= bits[c]
        y = 0
        for i in range(7):
            y = (y << 1) | bits[i]
        return y

    return np.array([fn(x) for x in range(128)], dtype=np.int64)


_P_R = _cnot_chain_perm()
_P_C = _P_R.copy()
_INV_PR = np.argsort(_P_R)
_INV_PC = np.argsort(_P_C)
_FLIP_MSB = I128 ^ 64
_BIT_LSB = I128 & 1
_MU = _P_C & 1  # parity of all 7 bits


def _order_rows_ring():
    return (I128 & 1) * 64 + (I128 >> 1)


def _order_cols_even_ring():
    return (I128 & 1) * 64 + (I128 >> 1)


def _order_cols_odd_ring():
    b7 = (I128 >> 6) & 1
    rest = I128 & 63
    mutilde = np.array([bin(int(r)).count("1") & 1 for r in rest])
    return b7 * 64 + mutilde * 32 + (rest >> 1)


_ORDER_V = ((I128 >> 6) & 1) * 64 + (I128 & 63)

# const-tile column offsets
_OFF_P1 = [i * 512 for i in range(4)]  # [A(256) | B(256)] per P1 pass
_OFF_P2 = [[2048 + (i * 2 + k) * 384 for k in (0, 1)] for i in range(4)]
_OFF_V = [5120 + k * 384 for k in (0, 1)]
_OFF_SIGN = 5888
_NCONST = 5890


def _pack_lhst(dst, off, L):
    """Pack [LiT | LrT | nLiT] (each 128 cols) for complex matmul L @ X."""
    dst[:, off : off + 128] = L.imag.T
    dst[:, off + 128 : off + 256] = L.real.T
    dst[:, off + 256 : off + 384] = -L.imag.T


def build_plan_consts(weight):
    """Returns the [128, _NCONST] float32 const tensor."""
    A, Bm = _layer_group_mats(weight)
    C = np.zeros((128, _NCONST), dtype=np.float64)

    cur_row_pos = I128.copy()
    cur_col_pos = I128.copy()
    pending_col = I128.copy()

    for l in range(4):
        if l % 2 == 0:
            new_row_pos = _order_rows_ring()
            lab_p = np.argsort(cur_row_pos)
            lab_j = np.argsort(new_row_pos)
            R = A[l][_INV_PR[lab_j][:, None], lab_p[None, :]].T
            C[:, _OFF_P1[l] : _OFF_P1[l] + 128] = R.real
            C[:, _OFF_P1[l] + 128 : _OFF_P1[l] + 256] = R.imag
            C[:, _OFF_P1[l] + 256 : _OFF_P1[l] + 384] = -R.imag
            C[:, _OFF_P1[l] + 384 : _OFF_P1[l] + 512] = R.real
            cur_row_pos = new_row_pos

            new_col_pos = _order_cols_even_ring()
            lab_p = np.argsort(cur_col_pos)
            lab_q = np.argsort(new_col_pos)
            in_idx = pending_col[lab_p]
            L0 = Bm[l][_INV_PC[lab_q][:, None], in_idx[None, :]]
            L1 = Bm[l][_FLIP_MSB[_INV_PC[lab_q]][:, None], in_idx[None, :]]
            _pack_lhst(C, _OFF_P2[l][0], L0)
            _pack_lhst(C, _OFF_P2[l][1], L1)
            cur_col_pos = new_col_pos
            pending_col = I128.copy()
        else:
            new_col_pos = _order_cols_odd_ring()
            lab_p = np.argsort(cur_col_pos)
            lab_j = np.argsort(new_col_pos)
            R = Bm[l][lab_j[:, None], lab_p[None, :]].T
            C[:, _OFF_P1[l] : _OFF_P1[l] + 128] = R.real
            C[:, _OFF_P1[l] + 128 : _OFF_P1[l] + 256] = R.imag
            C[:, _OFF_P1[l] + 256 : _OFF_P1[l] + 384] = -R.imag
            C[:, _OFF_P1[l] + 384 : _OFF_P1[l] + 512] = R.real
            cur_col_pos = new_col_pos

            new_row_pos = _order_rows_ring()
            lab_p = np.argsort(cur_row_pos)
            lab_q = np.argsort(new_row_pos)
            if l == 3:
                L0 = A[l][_INV_PR[lab_q][:, None], lab_p[None, :]]
                L1 = L0
            else:
                flip_mu0 = np.where(_BIT_LSB == 1, _FLIP_MSB, I128)
                flip_mu1 = np.where(_BIT_LSB == 0, _FLIP_MSB, I128)
                L0 = A[l][_INV_PR[flip_mu0[lab_q]][:, None], lab_p[None, :]]
                L1 = A[l][_INV_PR[flip_mu1[lab_q]][:, None], lab_p[None, :]]
            _pack_lhst(C, _OFF_P2[l][0], L0)
            _pack_lhst(C, _OFF_P2[l][1], L1)
            cur_row_pos = new_row_pos
            pending_col = _P_C.copy()

    lab_p = np.argsort(cur_row_pos)
    lab_q = np.argsort(_ORDER_V)
    V0 = A[4][lab_q[:, None], lab_p[None, :]]
    V1 = A[4][lab_q[:, None], _FLIP_MSB[lab_p][None, :]]
    _pack_lhst(C, _OFF_V[0], V0)
    _pack_lhst(C, _OFF_V[1], V1)

    C[:64, _OFF_SIGN] = 1.0
    C[64:, _OFF_SIGN] = -1.0
    return np.ascontiguousarray(C, dtype=np.float32)


# ------------------------------------------------------------- bass program

_PROGRAM_CACHE = {}


def build_program(
    n_samples=S,
    mm_dtype_name=MM_DTYPE,
    n_cores=N_CORES,
    debug_after=None,
    repeat=1,
):
    import concourse.bass as bass
    import concourse.mybir as mybir
    import concourse.tile as tile
    from concourse import bacc

    DT = getattr(mybir.dt, mm_dtype_name)
    F32 = mybir.dt.float32
    I32 = mybir.dt.int32
    pack2 = mm_dtype_name == "float16"  # 2-byte state: int32-pack swizzles
    ng = n_samples // 8  # 8-sample groups

    nc = bacc.Bacc(
        "TRN2",
        target_bir_lowering=False,
        debug=False,
        enable_asserts=False,
        num_devices=n_cores,
    )

    xr_d = nc.dram_tensor("xr", [n_samples, 16384], DT, kind="ExternalInput").ap()
    xi_d = nc.dram_tensor("xi", [n_samples, 16384], DT, kind="ExternalInput").ap()
    c_d = nc.dram_tensor("consts", [128, _NCONST], DT, kind="ExternalInput").ap()
    out_d = nc.dram_tensor("out", [n_samples, 1], F32, kind="ExternalOutput").ap()

    FREE = n_samples * 128
    if debug_after is not None:
        dbg_r = nc.dram_tensor(
            "dbg_r", [128, FREE], DT, kind="ExternalOutput"
        ).ap()
        dbg_i = nc.dram_tensor(
            "dbg_i", [128, FREE], DT, kind="ExternalOutput"
        ).ap()

    with tile.TileContext(nc) as tc:
        import contextlib

        ctx = contextlib.ExitStack()
        with ctx:
            singles = ctx.enter_context(tc.tile_pool(name="singles", bufs=1))
            psp = ctx.enter_context(tc.tile_pool(name="ps", bufs=4, space="PSUM"))
            scrp = ctx.enter_context(tc.tile_pool(name="scr", bufs=3))

            ctile = singles.tile([128, _NCONST], DT, tag="consts", name="ctile")
            # combined state: [Re | Im] per buffer
            X = [
                singles.tile([128, 2 * FREE], DT, tag="x0", name="x0"),
                singles.tile([128, 2 * FREE], DT, tag="x1", name="x1"),
            ]
            asum = singles.tile([128, n_samples], F32, tag="asum", name="asum")
            obuf = singles.tile([64, 1], F32, tag="obuf", name="obuf")
            sgn = singles.tile([128, 1], F32, tag="sgn", name="sgn")
            nc.gpsimd.memset(sgn[0:64], 1.0)
            nc.gpsimd.memset(sgn[64:128], -1.0)

            def planes(k):
                return X[k][:, :FREE], X[k][:, FREE:]

            # ---- load constants + input state
            nc.sync.dma_start(ctile[:], c_d)
            chunk = 8 if n_samples % 8 == 0 else 4
            X0r, X0i = planes(0)
            for c0 in range(0, n_samples, chunk):
                for src_d, dstp in ((xr_d, X0r), (xi_d, X0i)):
                    src = src_d[c0 : c0 + chunk, :].rearrange(
                        "s (r f) -> r s f", f=128
                    )
                    dst = dstp[:, c0 * 128 : (c0 + chunk) * 128].rearrange(
                        "p (s f) -> p s f", f=128
                    )
                    nc.sync.dma_start(dst, src)

            def mm(out_ap, lhsT, rhs, start, stop):
                if SKIP_MMS:
                    return
                nc.tensor.matmul(
                    out_ap,
                    lhsT.bitcast(DT),
                    rhs.bitcast(DT),
                    start=start,
                    stop=stop,
                )

            cp_engines = [
                lambda o, i: nc.vector.tensor_copy(o, i),
                lambda o, i: nc.scalar.copy(o, i),
            ]
            cp_idx = [0]

            def cp(o, i):
                if SKIP_COPIES:
                    return
                cp_engines[cp_idx[0] % 2](o, i)
                cp_idx[0] += 1

            def emit_p1_group(li, ksrc, kdst, g):
                sR, sI = planes(ksrc)
                base = _OFF_P1[li]
                cA = ctile[:, base : base + 256]
                cB = ctile[:, base + 256 : base + 512]
                if True:
                    ps = psp.tile([128, 1024], F32, tag="grp", name="psgrp")
                    for j in range(4):
                        s = 4 * g + j
                        sl = ps[:, 256 * j : 256 * j + 256]
                        mm(sl, sR[:, s * 128 : (s + 1) * 128], cA, True, False)
                        mm(sl, sI[:, s * 128 : (s + 1) * 128], cB, False, True)
                    # one merged-plane evict: psum (s, pl, 128) -> dst (pl, s, 128)
                    pv = ps[:].rearrange("p (s pl f) -> p pl s f", pl=2, f=128)
                    dv = X[kdst][:].rearrange(
                        "p (pl s f) -> p pl s f", pl=2, f=128
                    )[:, :, 4 * g : 4 * g + 4, :]
                    cp(dv, pv)

            def class_rhs(view_f128, view_x4, g, parity, k):
                if parity == 0:
                    return view_f128[:, 8 * g : 8 * g + 8, 64 * k : 64 * k + 64]
                if k == 0:
                    return view_x4[:, 8 * g : 8 * g + 8, ::3, :]
                return view_f128[:, 8 * g : 8 * g + 8, 32:96]

            def emit_p2_mms_class(ps, offs2, ksrc, g, parity, k):
                """One class-tile: ps = [ReC(512) | ImC(512)]. For the
                single-class case (parity-1 l=3), k indexes group halves."""
                sR, sI = planes(ksrc)
                if parity == 1 and offs2[0] == offs2[1]:
                    base = offs2[0]
                    sl = slice(1024 * g + 512 * k, 1024 * g + 512 * k + 512)
                    rr = sR[:, sl]
                    ri = sI[:, sl]
                else:
                    rf = [
                        v.rearrange("p (s f) -> p s f", f=128) for v in (sR, sI)
                    ]
                    r4 = [
                        v.rearrange("p (s x y) -> p s x y", x=4, y=32)
                        for v in (sR, sI)
                    ]
                    base = offs2[k]
                    rr = class_rhs(rf[0], r4[0], g, parity, k)
                    ri = class_rhs(rf[1], r4[1], g, parity, k)
                LiT = ctile[:, base : base + 128]
                LrT = ctile[:, base + 128 : base + 256]
                nLiT = ctile[:, base + 256 : base + 384]
                re_sl = ps[:, 0:512]
                im_sl = ps[:, 512:1024]
                mm(im_sl, LiT, rr, True, False)
                mm(im_sl, LrT, ri, False, True)
                mm(re_sl, LrT, rr, True, False)
                mm(re_sl, nLiT, ri, False, True)

            def evict_p2_class(ps, g, parity, nclass, kdst, k):
                # ps: [ReC(8s x 64) | ImC(8s x 64)] for class k (or half k
                # of the single-class l=3 case).
                gs = slice(8 * g, 8 * g + 8)
                D = X[kdst][:].rearrange("p (pl s f) -> p pl s f", pl=2, f=128)[
                    :, :, gs, :
                ]
                D4 = X[kdst][:].rearrange(
                    "p (pl s x y) -> p pl s x y", pl=2, x=4, y=32
                )[:, :, gs, :, :]
                pv = ps[:].rearrange("p (pl s f) -> p pl s f", pl=2, f=64)
                ph = ps[:].rearrange(
                    "p (pl s x y) -> p pl s x y", pl=2, x=2, y=32
                )
                if nclass == 1:
                    # half k covers samples 4k..4k+3 of the group, full 128 f
                    hs = slice(8 * g + 4 * k, 8 * g + 4 * k + 4)
                    Dh = X[kdst][:].rearrange(
                        "p (pl s f) -> p pl s f", pl=2, f=128
                    )[:, :, hs, :]
                    d2 = X[kdst][:].rearrange(
                        "p (pl s x y) -> p pl s x y", pl=2, x=2, y=64
                    )[:, :, hs, :, :]
                    pvh = ps[:].rearrange("p (pl s f) -> p pl s f", pl=2, f=128)
                    p2 = ps[:].rearrange(
                        "p (pl s x y) -> p pl s x y", pl=2, x=2, y=64
                    )
                    cp(Dh[0:64], pvh[0:64])
                    cp(d2[64:128, :, :, 1, :], p2[64:128, :, :, 0, :])
                    cp(d2[64:128, :, :, 0, :], p2[64:128, :, :, 1, :])
                    return
                if parity == 0:
                    cp(D[0:64, :, :, 64 * k : 64 * k + 64], pv[0:64])
                    cp(D4[64:128, :, :, 2 * k + 1, :], ph[64:128, :, :, 0, :])
                    cp(D4[64:128, :, :, 2 * k, :], ph[64:128, :, :, 1, :])
                else:
                    if k == 0:
                        cp(D4[0:64, :, :, ::3, :], ph[0:64])
                        cp(D4[64:128, :, :, 2, :], ph[64:128, :, :, 0, :])
                        cp(D4[64:128, :, :, 1, :], ph[64:128, :, :, 1, :])
                    else:
                        cp(D[0:64, :, :, 32:96], pv[0:64])
                        cp(D4[64:128, :, :, 3, :], ph[64:128, :, :, 0, :])
                        cp(D4[64:128, :, :, 0, :], ph[64:128, :, :, 1, :])

            def gcp(o, i):
                # gpsimd swizzle; int32-pack halves the element count for
                # 2-byte state dtypes (innermost runs are 32+ elems, 4B-aligned)
                if pack2:
                    o, i = o.bitcast(I32), i.bitcast(I32)
                nc.gpsimd.tensor_copy(o, i)

            def gps_place_class(scr, g, parity, nclass, kdst, k):
                # same placement patterns as evict_p2_class, but reading the
                # scratch copy and with the upper-half block swaps folded into
                # single negative-step views.
                gs = slice(8 * g, 8 * g + 8)
                D = X[kdst][:].rearrange("p (pl s f) -> p pl s f", pl=2, f=128)[
                    :, :, gs, :
                ]
                D4 = X[kdst][:].rearrange(
                    "p (pl s x y) -> p pl s x y", pl=2, x=4, y=32
                )[:, :, gs, :, :]
                sc = scr[:, 1024 * k : 1024 * k + 1024]
                sv = sc.rearrange("p (pl s f) -> p pl s f", pl=2, f=64)
                sh = sc.rearrange("p (pl s x y) -> p pl s x y", pl=2, x=2, y=32)
                if nclass == 1:
                    hs = slice(8 * g + 4 * k, 8 * g + 4 * k + 4)
                    Dh = X[kdst][:].rearrange(
                        "p (pl s f) -> p pl s f", pl=2, f=128
                    )[:, :, hs, :]
                    d2 = X[kdst][:].rearrange(
                        "p (pl s x y) -> p pl s x y", pl=2, x=2, y=64
                    )[:, :, hs, :, :]
                    svh = sc.rearrange("p (pl s f) -> p pl s f", pl=2, f=128)
                    s2 = sc.rearrange("p (pl s x y) -> p pl s x y", pl=2, x=2, y=64)
                    gcp(Dh[0:64], svh[0:64])
                    # dst x (1,0) <- src x (0,1)
                    gcp(d2[64:128, :, :, 1::-1, :], s2[64:128])
                    return
                if parity == 0:
                    gcp(D[0:64, :, :, 64 * k : 64 * k + 64], sv[0:64])
                    # dst x (2k+1, 2k) <- src x (0, 1)
                    xs = slice(2 * k + 1, (2 * k - 1) if k > 0 else None, -1)
                    gcp(D4[64:128, :, :, xs, :], sh[64:128])
                else:
                    if k == 0:
                        gcp(D4[0:64, :, :, ::3, :], sh[0:64])
                        # dst x (2,1) <- src x (0,1)
                        gcp(D4[64:128, :, :, 2:0:-1, :], sh[64:128])
                    else:
                        gcp(D[0:64, :, :, 32:96], sv[0:64])
                        # dst x (3,0) <- src x (0,1)
                        gcp(D4[64:128, :, :, 3::-3, :], sh[64:128])

            def emit_p2_group(li, ksrc, kdst, g):
                parity = li % 2
                nclass = 1 if li == 3 else 2
                offs2 = _OFF_P2[li]
                if nclass == 1:
                    offs2 = [offs2[0], offs2[0]]
                if GPS:
                    scr = scrp.tile([128, 2048], DT, tag="scr", name="scr")
                    for k in range(2):
                        ps = psp.tile([128, 1024], F32, tag="grp", name="psgrp")
                        emit_p2_mms_class(ps, offs2, ksrc, g, parity, k)
                        cp(scr[:, 1024 * k : 1024 * k + 1024], ps[:])
                    for k in range(2):
                        gps_place_class(scr, g, parity, nclass, kdst, k)
                    return
                for k in range(2):
                    ps = psp.tile([128, 1024], F32, tag="grp", name="psgrp")
                    emit_p2_mms_class(ps, offs2, ksrc, g, parity, k)
                    evict_p2_class(ps, g, parity, nclass, kdst, k)

            def emit_p1(li, ksrc, kdst):
                for g in range(2 * ng):
                    emit_p1_group(li, ksrc, kdst, g)

            def emit_p2(li, ksrc, kdst):
                for g in range(ng):
                    emit_p2_group(li, ksrc, kdst, g)

            def emit_layer_interleaved(li, a, b):
                for g in range(ng):
                    emit_p1_group(li, a, b, 2 * g)
                    emit_p1_group(li, a, b, 2 * g + 1)
                    emit_p2_group(li, b, a, g)

            # ---- the 8 layer passes (+ V), optionally in a HW loop
            state = {"done": 0, "tail": True}
            stop = 9 if debug_after is None else debug_after

            def emit_main():
                done = state["done"]
                for li in range(4):
                    if INTERLEAVE and done + 2 <= stop:
                        emit_layer_interleaved(li, 0, 1)
                        done += 2
                        continue
                    if done < stop:
                        emit_p1(li, 0, 1)
                        done += 1
                        if done == stop:
                            nc.sync.dma_start(dbg_r, X[1][:, :FREE])
                            nc.sync.dma_start(dbg_i, X[1][:, FREE:])
                    if done < stop:
                        emit_p2(li, 1, 0)
                        done += 1
                        if done == stop:
                            nc.sync.dma_start(dbg_r, X[0][:, :FREE])
                            nc.sync.dma_start(dbg_i, X[0][:, FREE:])
                tail = done < stop
                # ---- V pass (reads buf 0, squares into buf 1 scratch)
                sv2 = X[1][:].rearrange(
                    "p (s k2 f) -> p k2 s f", k2=4, f=64
                )
                for g in range(ng if tail else 0):
                    for k in range(2):
                        ps = psp.tile([128, 1024], F32, tag="grp", name="psgrp")
                        emit_p2_mms_class(ps, _OFF_V, 0, g, parity=1, k=k)
                        # evict = square: ps (pl, s, 64) -> scratch k2-slots
                        pv = ps[:].rearrange("p (pl s f) -> p pl s f", pl=2, f=64)
                        nc.scalar.activation(
                            sv2[:, k::2, 8 * g : 8 * g + 8, :],
                            pv,
                            bass.mybir.ActivationFunctionType.Square,
                        )
                if tail:
                    xq = X[1][:].rearrange("p (s q) -> p s q", q=256)
                    for g in range(ng):
                        gs = slice(8 * g, 8 * g + 8)
                        nc.vector.tensor_reduce(
                            asum[:, gs].unsqueeze(-1),
                            xq[:, gs, :],
                            axis=bass.mybir.AxisListType.X,
                            op=bass.mybir.AluOpType.add,
                        )
                state["done"] = done
                state["tail"] = tail

            if repeat > 1:
                with tc.For_i(0, repeat, 1):
                    emit_main()
            else:
                emit_main()
            tail = state["tail"]

            # ---- final signed reduce: out[s] = sum_p sign[p] * asum[p, s]
            if tail:
                psfin = psp.tile([n_samples, 1], F32, tag="grp", name="psfin")
                nc.tensor.matmul(
                    psfin[:],
                    asum[:].bitcast(F32),
                    sgn[:].bitcast(F32),
                    start=True,
                    stop=True,
                )
                nc.vector.tensor_copy(obuf[0:n_samples], psfin[:])
                nc.sync.dma_start(out_d, obuf[0:n_samples])
            else:
                nc.vector.memset(obuf[:], 0.0)
                nc.sync.dma_start(out_d, obuf[0:n_samples])

    nc.compile()
    return nc


# ----------------------------------------------------------------- entry


def round_fp32r(a):
    """Round-to-nearest-even to 11 mantissa bits (FP32R)."""
    u = np.ascontiguousarray(a, dtype=np.float32).view(np.uint32)
    r = (u + np.uint32(0x7FF) + ((u >> np.uint32(12)) & np.uint32(1))) & np.uint32(
        0xFFFFF000
    )
    return r.view(np.float32)


def kernel(x_real, x_imag, weight):
    from concourse.bass_utils import run_bass_kernel_spmd

    key = (S, MM_DTYPE, N_CORES)
    if key not in _PROGRAM_CACHE:
        _PROGRAM_CACHE[key] = build_program(S, MM_DTYPE, N_CORES)
    nc = _PROGRAM_CACHE[key]

    consts = build_plan_consts(np.asarray(weight, dtype=np.float64))
    xr = np.ascontiguousarray(
        np.asarray(x_real, dtype=np.float32).reshape(B_FULL, 16384)
    )
    xi = np.ascontiguousarray(
        np.asarray(x_imag, dtype=np.float32).reshape(B_FULL, 16384)
    )
    if MM_DTYPE == "float16":
        consts = consts.astype(np.float16)
        xr = xr.astype(np.float16)
        xi = xi.astype(np.float16)
    if MM_DTYPE == "float32r":
        consts = round_fp32r(consts)
        xr = round_fp32r(xr)
        xi = round_fp32r(xi)

    in_maps = []
    for c in range(N_CORES):
        in_maps.append(
            {
                "xr": xr[c * S : (c + 1) * S],
                "xi": xi[c * S : (c + 1) * S],
                "consts": consts,
            }
        )

    trace = os.environ.get("CIR_TRACE", "") == "1"
    res = run_bass_kernel_spmd(
        nc, in_maps, core_ids=list(range(N_CORES)), trace=trace
    )
    LAST_RESULTS[0] = res
    out = np.concatenate([r["out"] for r in res.results], axis=0)
    return out.astype(np.float32)


LAST_RESULTS = [None]

